# revision 2
# baseline (speedup 1.0000x reference)
"""HarsanyiNet forward on 8 TRN2 NeuronCores — single-launch design.

Structure (vs the 2-launch baseline): ONE kernel launch computes both
layers, eliminating the second launch's ~10us NRT preamble/postamble and
the host bounce.  No cross-core exchange exists in this environment
(remote_dma ucode is absent from the terminal runtime; a cc AllGather
measures ~90us), so layer 0 is computed IN FULL on every core
(replicated weights m0/w0), and layer 1 + both heads are sharded by
hidden-output chunk.

Numerics (validated in numpy against the jax reference, rel err ~3e-3
vs the 2e-2 gate):
  * L = log(tanh(G|x|)) = ln(1-z) - ln(1+z), z = exp(-2G|x| - 1e-6)
    -- all of {exp, ln} in ONE ScalarE table set (single table load).
    The -1e-6 bias keeps z < 1 so ln(1-z) >= ~-13.8 (a masked x==0
    entry contributes e^-13.8 per factor instead of an exact 0 -- far
    below the output scale).  A -30000 clamp guards LUT rounding.
  * Single-bf16 everywhere: L, x, folded weights w = bf16(fc*(v>0)),
    heads bf16, h0 cast to bf16 before layer 1.  No hi/lo splits.
  * head0 is pre-scaled by 1/8 on the host: every core computes the
    FULL y0 = h0 @ head0.T/8, so the host's sum over 8 per-core partial
    outputs reconstructs y0 exactly once.

Layouts: feature-major [feature, batch], 1024-long dims pre-split into
8 chunk-major blocks of 128 partitions.  Weight stationaries are packed
[128, k*... ] so every LDWEIGHTS operand is a natural column block.
"""
import sys

import numpy as np

sys.path.insert(0, "/opt/trn_rl_repo")

import ml_dtypes  # noqa: E402

from concourse import bacc, mybir, tile  # noqa: E402
from concourse.alu_op_type import AluOpType  # noqa: E402
from concourse.bass_utils import run_bass_kernel_spmd  # noqa: E402
from concourse.tile_rust import add_dep_helper  # noqa: E402


def _order(after, before, why):
    add_dep_helper(getattr(after, "ins", after), getattr(before, "ins", before),
                   sync=False, reason=why)

B, NIN, HID, C = 64, 1024, 1024, 10
GAMMA = 100.0
N_CORES = 8
OSH = HID // N_CORES        # 128 rows of layer-1 output per core
KCH = NIN // 128            # 8 contraction chunks
KB = KCH * B                # 512 activation columns, chunk-major
LCLAMP = -30000.0
F32 = mybir.dt.float32
BF16 = mybir.dt.bfloat16
BF16_NP = ml_dtypes.bfloat16
NH = 2                      # column-half pipelining for elementwise chains
HB = KB // NH               # 256

PROFILE = {"enable": False, "trace_kwargs": {}, "runs": []}
_CACHE = {}


def _force_act_table_set(target="natural_log_exp_and_others"):
    import concourse.bacc as bacc_mod
    from concourse.hw_specs import get_activation_tables as real_tabs

    def patched(arch):
        tabs = real_tabs(arch)
        return {name: (funcs if name == target else set())
                for name, funcs in tabs.items()}

    bacc_mod.get_activation_tables = patched


def _build():
    _force_act_table_set()
    nc = bacc.Bacc("TRN2", target_bir_lowering=False, debug=False,
                   num_devices=N_CORES, enable_asserts=False)
    Act = mybir.ActivationFunctionType
    # --- inputs (per-core maps supplied by the host) ---
    xT = nc.declare_dram_parameter("xT", [128, KB], BF16, isOutput=False)
    m0f = nc.declare_dram_parameter("m0f", [128, KCH * HID], BF16, isOutput=False)
    w0f = nc.declare_dram_parameter("w0f", [128, KCH * HID], BF16, isOutput=False)
    # tail pack: m1s (KCH*OSH) | w1s (KCH*OSH) | hd0f (KCH*C) | hd1s (C)
    TAILC = KCH * OSH * 2 + KCH * C + C
    tail = nc.declare_dram_parameter("tail", [128, TAILC], BF16, isOutput=False)
    y_part = nc.declare_dram_parameter("y_part", [C, B], F32, isOutput=True)
    import os
    dbg = bool(os.environ.get("K2_DEBUG"))
    if dbg:
        Lb_o = nc.declare_dram_parameter("Lb_o", [128, KB], BF16, isOutput=True)
        h0_o = nc.declare_dram_parameter("h0_o", [128, KCH * B], BF16, isOutput=True)
        S0_o = nc.declare_dram_parameter("S0_o", [128, KCH * B], F32, isOutput=True)
        HL0_o = nc.declare_dram_parameter("HL0_o", [128, KCH * B], F32, isOutput=True)

    with tile.TileContext(nc) as tc:
        with (
            tc.tile_pool(name="sb", bufs=1) as sb,
            tc.tile_pool(name="ps", bufs=1, space="PSUM") as ps,
        ):
            # --- input DMAs, ordered by first use ---
            xt = sb.tile([128, KB], BF16)
            for h in range(NH):
                nc.sync.dma_start(xt[:, h * HB:(h + 1) * HB],
                                  xT[:, h * HB:(h + 1) * HB])
            m0t = sb.tile([128, KCH * HID], BF16)
            for q in range(4):      # k-pair slabs so S0 can start early
                s = slice(q * 2 * HID, (q + 1) * 2 * HID)
                nc.sync.dma_start(m0t[:, s], m0f[:, s])
            w0t = sb.tile([128, KCH * HID], BF16)
            for q in range(4):
                s = slice(q * 2 * HID, (q + 1) * 2 * HID)
                nc.sync.dma_start(w0t[:, s], w0f[:, s])
            tlt = sb.tile([128, TAILC], BF16)
            nc.sync.dma_start(tlt[:], tail[:, :])
            m1t = tlt[:, 0:KCH * OSH]
            w1t = tlt[:, KCH * OSH:2 * KCH * OSH]
            hd0t = tlt[:, 2 * KCH * OSH:2 * KCH * OSH + KCH * C]
            hd1t = tlt[:, 2 * KCH * OSH + KCH * C:]

            eps = sb.tile([128, 1], F32)
            nc.vector.memset(eps[:], -1e-6)

            # --- L0 chain on full x, two column halves ---
            a = sb.tile([128, KB], F32)
            z = sb.tile([128, KB], F32)
            p = sb.tile([128, KB], F32)
            q_ = sb.tile([128, KB], F32)
            Lb = sb.tile([128, KB], BF16)
            for h in range(NH):
                cs = slice(h * HB, (h + 1) * HB)
                nc.vector.scalar_tensor_tensor(a[:, cs], xt[:, cs], -1.0,
                                               xt[:, cs],
                                               op0=AluOpType.mult,
                                               op1=AluOpType.max)
                nc.scalar.activation(z[:, cs], a[:, cs], Act.Exp,
                                     scale=-2.0 * GAMMA, bias=eps[:])
                nc.scalar.activation(p[:, cs], z[:, cs], Act.Ln,
                                     bias=1.0, scale=-1.0)
                nc.scalar.activation(q_[:, cs], z[:, cs], Act.Ln,
                                     bias=1.0, scale=1.0)
                nc.vector.scalar_tensor_tensor(Lb[:, cs], p[:, cs], LCLAMP,
                                               q_[:, cs],
                                               op0=AluOpType.max,
                                               op1=AluOpType.subtract)

            # --- layer-0 full matmuls: S0 and HL0 over all 8 out-chunks ---
            S0 = ps.tile([128, KCH * B], F32)   # out-chunk o at cols o*B
            HL0 = ps.tile([128, KCH * B], F32)
            # o-outer / k-inner: each PSUM region's accumulation group is
            # contiguous (interleaved starts within a bank corrupt results).
            for o in range(KCH):
                for k in range(KCH):
                    nc.tensor.matmul(
                        S0[:, o * B:(o + 1) * B],
                        m0t[:, (o * KCH + k) * 128:(o * KCH + k + 1) * 128],
                        Lb[:, k * B:(k + 1) * B],
                        start=(k == 0), stop=(k == KCH - 1))
            for o in range(KCH):
                for k in range(KCH):
                    nc.tensor.matmul(
                        HL0[:, o * B:(o + 1) * B],
                        w0t[:, (o * KCH + k) * 128:(o * KCH + k + 1) * 128],
                        xt[:, k * B:(k + 1) * B],
                        start=(k == 0), stop=(k == KCH - 1))

            # --- h0 = relu(HL0)*exp(S0), cast straight to bf16; L1 chain ---
            d0 = sb.tile([128, KCH * B], F32)
            h0b = sb.tile([128, KCH * B], BF16)
            z1 = sb.tile([128, KB], F32)
            p1 = sb.tile([128, KB], F32)
            q1 = sb.tile([128, KB], F32)
            L1b = sb.tile([128, KB], BF16)
            for h in range(NH):
                cs = slice(h * HB, (h + 1) * HB)
                nc.scalar.activation(d0[:, cs], S0[:, cs], Act.Exp)
                nc.vector.scalar_tensor_tensor(h0b[:, cs], HL0[:, cs], 0.0,
                                               d0[:, cs],
                                               op0=AluOpType.max,
                                               op1=AluOpType.mult)
                # L1 = lntanh(G*h0), h0 >= 0 so no abs needed
                nc.scalar.activation(z1[:, cs], h0b[:, cs], Act.Exp,
                                     scale=-2.0 * GAMMA, bias=eps[:])
                nc.scalar.activation(p1[:, cs], z1[:, cs], Act.Ln,
                                     bias=1.0, scale=-1.0)
                nc.scalar.activation(q1[:, cs], z1[:, cs], Act.Ln,
                                     bias=1.0, scale=1.0)
                nc.vector.scalar_tensor_tensor(L1b[:, cs], p1[:, cs], LCLAMP,
                                               q1[:, cs],
                                               op0=AluOpType.max,
                                               op1=AluOpType.subtract)

            # --- y0 (full, head0 pre-scaled 1/8) + layer-1 sharded ---
            # Strictly sequential PSUM accumulation groups (no group spans
            # another); head1 gets its own tile, summed on DVE.
            Y = ps.tile([C, B], F32)
            for k in range(KCH):
                nc.tensor.matmul(Y[:], hd0t[:, k * C:(k + 1) * C],
                                 h0b[:, k * B:(k + 1) * B],
                                 start=(k == 0), stop=(k == KCH - 1))
            S1 = ps.tile([OSH, B], F32)
            HL1 = ps.tile([OSH, B], F32)
            for k in range(KCH):
                nc.tensor.matmul(HL1[:], w1t[:, k * OSH:(k + 1) * OSH],
                                 h0b[:, k * B:(k + 1) * B],
                                 start=(k == 0), stop=(k == KCH - 1))
            for k in range(KCH):
                nc.tensor.matmul(S1[:], m1t[:, k * OSH:(k + 1) * OSH],
                                 L1b[:, k * B:(k + 1) * B],
                                 start=(k == 0), stop=(k == KCH - 1))

            if dbg:
                nc.sync.dma_start(Lb_o[:, :], Lb[:])
                nc.sync.dma_start(h0_o[:, :], h0b[:])
                S0c = sb.tile([128, KCH * B], F32)
                nc.vector.tensor_copy(S0c[:], S0[:])
                nc.sync.dma_start(S0_o[:, :], S0c[:])
                HL0c = sb.tile([128, KCH * B], F32)
                nc.vector.tensor_copy(HL0c[:], HL0[:])
                nc.sync.dma_start(HL0_o[:, :], HL0c[:])
            d1 = sb.tile([OSH, B], F32)
            nc.scalar.activation(d1[:], S1[:], Act.Exp)
            h1b = sb.tile([OSH, B], BF16)
            nc.vector.scalar_tensor_tensor(h1b[:], HL1[:], 0.0, d1[:],
                                           op0=AluOpType.max,
                                           op1=AluOpType.mult)
            Y1 = ps.tile([C, B], F32)
            nc.tensor.matmul(Y1[:], hd1t[:], h1b[:], start=True, stop=True)
            yo = sb.tile([C, B], F32)
            nc.vector.tensor_copy(yo[:], Y[:])
            nc.vector.tensor_tensor(yo[:], yo[:], Y1[:], op=AluOpType.add)
            nc.sync.dma_start(y_part[:, :], yo[:])
    nc.compile()
    return nc


def _omajor_blocks(m: np.ndarray) -> np.ndarray:
    """[1024 out, 1024 in] -> [128, 64*128]: lhsT block (o,k) at col
    (o*8+k)*128; arr[p, (o*8+k)*128+j] = m[o*128+j, k*128+p]."""
    return np.ascontiguousarray(
        m.reshape(KCH, 128, KCH, 128).transpose(3, 0, 2, 1).reshape(128, -1))


def _chunk_major(mat_t: np.ndarray) -> np.ndarray:
    """[1024, cols] -> [128, 8*cols]: row-chunk k lands at col offset k*cols."""
    rows, cols = mat_t.shape
    assert rows == KCH * 128
    return np.ascontiguousarray(
        mat_t.reshape(KCH, 128, cols).transpose(1, 0, 2).reshape(128, KCH * cols)
    )


def kernel(x, v0, fc0, head0, v1, fc1, head1):
    nc = _CACHE.get("nc")
    if nc is None:
        nc = _CACHE["nc"] = _build()

    x = np.asarray(x, np.float32)
    m0 = (np.asarray(v0) > 0).astype(np.float32)
    w0 = (np.asarray(fc0, np.float32) * m0).astype(BF16_NP)
    m1 = (np.asarray(v1) > 0).astype(np.float32)
    w1 = (np.asarray(fc1, np.float32) * m1).astype(BF16_NP)

    xT = _chunk_major(np.ascontiguousarray(x.T)).astype(BF16_NP)
    m0f = _omajor_blocks(m0).astype(BF16_NP)
    w0f = _omajor_blocks(w0.astype(np.float32)).astype(BF16_NP)
    hd0f = _chunk_major(
        np.ascontiguousarray((np.asarray(head0, np.float32) / N_CORES).T)
    ).astype(BF16_NP)

    in_maps = []
    for c in range(N_CORES):
        sl = slice(c * OSH, (c + 1) * OSH)
        m1s = _chunk_major(np.ascontiguousarray(m1[sl].T)).astype(BF16_NP)
        w1s = _chunk_major(
            np.ascontiguousarray(w1[sl].T.astype(np.float32))).astype(BF16_NP)
        hd1s = np.ascontiguousarray(
            np.asarray(head1, np.float32)[:, sl].T).astype(BF16_NP)
        tail = np.concatenate([m1s, w1s, hd0f, hd1s], axis=1)
        in_maps.append({"xT": xT, "m0f": m0f, "w0f": w0f, "tail": tail})

    kwargs = {}
    if PROFILE["enable"]:
        kwargs = {"trace": True, **PROFILE["trace_kwargs"]}
    res = run_bass_kernel_spmd(nc, in_maps, core_ids=list(range(N_CORES)),
                               **kwargs)
    if PROFILE["enable"]:
        PROFILE["runs"].append(res)
    y = np.zeros((C, B), np.float32)
    for c in range(N_CORES):
        y += res.results[c]["y_part"]
    return np.ascontiguousarray(y.T).astype(np.float32)


# revision 3
# speedup vs baseline: 1.0694x; 1.0694x over previous
"""HarsanyiNet forward on 8 TRN2 NeuronCores — single-launch design.

Structure (vs the 2-launch baseline): ONE kernel launch computes both
layers, eliminating the second launch's ~10us NRT preamble/postamble and
the host bounce.  No cross-core exchange exists in this environment
(remote_dma ucode is absent from the terminal runtime; a cc AllGather
measures ~90us), so layer 0 is computed IN FULL on every core
(replicated weights m0/w0), and layer 1 + both heads are sharded by
hidden-output chunk.

Numerics (validated in numpy against the jax reference, rel err ~3e-3
vs the 2e-2 gate):
  * L = log(tanh(G|x|)) = ln(1-z) - ln(1+z), z = exp(-2G|x| - 1e-6)
    -- all of {exp, ln} in ONE ScalarE table set (single table load).
    The -1e-6 bias keeps z < 1 so ln(1-z) >= ~-13.8 (a masked x==0
    entry contributes e^-13.8 per factor instead of an exact 0 -- far
    below the output scale).  A -30000 clamp guards LUT rounding.
  * Single-bf16 everywhere: L, x, folded weights w = bf16(fc*(v>0)),
    heads bf16, h0 cast to bf16 before layer 1.  No hi/lo splits.
  * head0 is pre-scaled by 1/8 on the host: every core computes the
    FULL y0 = h0 @ head0.T/8, so the host's sum over 8 per-core partial
    outputs reconstructs y0 exactly once.

Layouts: feature-major [feature, batch], 1024-long dims pre-split into
8 chunk-major blocks of 128 partitions.  Weight stationaries are packed
[128, k*... ] so every LDWEIGHTS operand is a natural column block.
"""
import sys

import numpy as np

sys.path.insert(0, "/opt/trn_rl_repo")

import ml_dtypes  # noqa: E402

from concourse import bacc, mybir, tile  # noqa: E402
from concourse.alu_op_type import AluOpType  # noqa: E402
from concourse.bass_utils import run_bass_kernel_spmd  # noqa: E402
from concourse.tile_rust import add_dep_helper  # noqa: E402


def _order(after, before, why):
    add_dep_helper(getattr(after, "ins", after), getattr(before, "ins", before),
                   sync=False, reason=why)

B, NIN, HID, C = 64, 1024, 1024, 10
GAMMA = 100.0
N_CORES = 8
OSH = HID // N_CORES        # 128 rows of layer-1 output per core
KCH = NIN // 128            # 8 contraction chunks
KB = KCH * B                # 512 activation columns, chunk-major
LCLAMP = -30000.0
F32 = mybir.dt.float32
BF16 = mybir.dt.bfloat16
BF16_NP = ml_dtypes.bfloat16
NH = 2                      # column-half pipelining for elementwise chains
HB = KB // NH               # 256

PROFILE = {"enable": False, "trace_kwargs": {}, "runs": []}
_CACHE = {}


def _force_act_table_set(target="natural_log_exp_and_others"):
    import concourse.bacc as bacc_mod
    from concourse.hw_specs import get_activation_tables as real_tabs

    def patched(arch):
        tabs = real_tabs(arch)
        return {name: (funcs if name == target else set())
                for name, funcs in tabs.items()}

    bacc_mod.get_activation_tables = patched


def _build():
    _force_act_table_set()
    nc = bacc.Bacc("TRN2", target_bir_lowering=False, debug=False,
                   num_devices=N_CORES, enable_asserts=False)
    Act = mybir.ActivationFunctionType
    # --- inputs (per-core maps supplied by the host) ---
    xT = nc.declare_dram_parameter("xT", [128, KB], BF16, isOutput=False)
    m0f = nc.declare_dram_parameter("m0f", [128, KCH * HID], BF16, isOutput=False)
    w0f = nc.declare_dram_parameter("w0f", [128, KCH * HID], BF16, isOutput=False)
    # tail pack: m1s (KCH*OSH) | w1s (KCH*OSH) | hd0f (KCH*C) | hd1s (C)
    TAILC = KCH * OSH * 2 + KCH * C + C
    tail = nc.declare_dram_parameter("tail", [128, TAILC], BF16, isOutput=False)
    y_part = nc.declare_dram_parameter("y_part", [C, B], F32, isOutput=True)
    import os
    dbg = bool(os.environ.get("K2_DEBUG"))
    if dbg:
        Lb_o = nc.declare_dram_parameter("Lb_o", [128, KB], BF16, isOutput=True)
        h0_o = nc.declare_dram_parameter("h0_o", [128, KCH * B], BF16, isOutput=True)
        S0_o = nc.declare_dram_parameter("S0_o", [128, KCH * B], F32, isOutput=True)
        HL0_o = nc.declare_dram_parameter("HL0_o", [128, KCH * B], F32, isOutput=True)

    with tile.TileContext(nc) as tc:
        with (
            tc.tile_pool(name="sb", bufs=1) as sb,
            tc.tile_pool(name="ps", bufs=1, space="PSUM") as ps,
        ):
            # --- input DMAs, ordered by first use ---
            xt = sb.tile([128, KB], BF16)
            for h in range(NH):
                nc.sync.dma_start(xt[:, h * HB:(h + 1) * HB],
                                  xT[:, h * HB:(h + 1) * HB])
            m0t = sb.tile([128, KCH * HID], BF16)
            for q in range(4):      # k-pair slabs so S0 can start early
                s = slice(q * 2 * HID, (q + 1) * 2 * HID)
                nc.sync.dma_start(m0t[:, s], m0f[:, s])
            w0t = sb.tile([128, KCH * HID], BF16)
            for q in range(4):
                s = slice(q * 2 * HID, (q + 1) * 2 * HID)
                nc.sync.dma_start(w0t[:, s], w0f[:, s])
            tlt = sb.tile([128, TAILC], BF16)
            nc.sync.dma_start(tlt[:], tail[:, :])
            m1t = tlt[:, 0:KCH * OSH]
            w1t = tlt[:, KCH * OSH:2 * KCH * OSH]
            hd0t = tlt[:, 2 * KCH * OSH:2 * KCH * OSH + KCH * C]
            hd1t = tlt[:, 2 * KCH * OSH + KCH * C:]

            eps = sb.tile([128, 1], F32)
            nc.vector.memset(eps[:], -1e-6)

            # --- L0 chain on full x, two column halves ---
            a = sb.tile([128, KB], F32)
            z = sb.tile([128, KB], F32)
            p = sb.tile([128, KB], F32)
            q_ = sb.tile([128, KB], F32)
            Lb = sb.tile([128, KB], BF16)
            for h in range(NH):
                cs = slice(h * HB, (h + 1) * HB)
                nc.vector.scalar_tensor_tensor(a[:, cs], xt[:, cs], -1.0,
                                               xt[:, cs],
                                               op0=AluOpType.mult,
                                               op1=AluOpType.max)
                nc.scalar.activation(z[:, cs], a[:, cs], Act.Exp,
                                     scale=-2.0 * GAMMA, bias=eps[:])
                nc.scalar.activation(p[:, cs], z[:, cs], Act.Ln,
                                     bias=1.0, scale=-1.0)
                nc.scalar.activation(q_[:, cs], z[:, cs], Act.Ln,
                                     bias=1.0, scale=1.0)
                nc.vector.scalar_tensor_tensor(Lb[:, cs], p[:, cs], LCLAMP,
                                               q_[:, cs],
                                               op0=AluOpType.max,
                                               op1=AluOpType.subtract)

            # --- layer-0 full matmuls: S0 and HL0 over all 8 out-chunks ---
            # Each half (out-chunks 0-3 / 4-7) gets its OWN PSUM tile: the
            # downstream delta0/h0/L1 half-chains then carry honest
            # whole-tile deps and start as soon as THEIR half's groups
            # close, instead of waiting for all 8 groups of a shared tile.
            S0h = [ps.tile([128, 4 * B], F32, name=f"S0h{i}") for i in range(2)]
            HL0h = [ps.tile([128, 4 * B], F32, name=f"HL0h{i}") for i in range(2)]
            # o-outer / k-inner: each PSUM region's accumulation group is
            # contiguous (interleaved starts within a bank corrupt results).
            for o in range(KCH):
                dst = S0h[o // 4][:, (o % 4) * B:(o % 4 + 1) * B]
                for k in range(KCH):
                    nc.tensor.matmul(
                        dst,
                        m0t[:, (o * KCH + k) * 128:(o * KCH + k + 1) * 128],
                        Lb[:, k * B:(k + 1) * B],
                        start=(k == 0), stop=(k == KCH - 1))
            for o in range(KCH):
                dst = HL0h[o // 4][:, (o % 4) * B:(o % 4 + 1) * B]
                for k in range(KCH):
                    nc.tensor.matmul(
                        dst,
                        w0t[:, (o * KCH + k) * 128:(o * KCH + k + 1) * 128],
                        xt[:, k * B:(k + 1) * B],
                        start=(k == 0), stop=(k == KCH - 1))

            # --- h0 = relu(HL0)*exp(S0), cast straight to bf16; L1 chain ---
            d0 = sb.tile([128, KCH * B], F32)
            h0b = sb.tile([128, KCH * B], BF16)
            z1 = sb.tile([128, KB], F32)
            p1 = sb.tile([128, KB], F32)
            q1 = sb.tile([128, KB], F32)
            L1b = sb.tile([128, KB], BF16)
            for h in range(NH):
                cs = slice(h * HB, (h + 1) * HB)
                nc.scalar.activation(d0[:, cs], S0h[h][:], Act.Exp)
                nc.vector.scalar_tensor_tensor(h0b[:, cs], HL0h[h][:], 0.0,
                                               d0[:, cs],
                                               op0=AluOpType.max,
                                               op1=AluOpType.mult)
                # L1 = lntanh(G*h0), h0 >= 0 so no abs needed
                nc.scalar.activation(z1[:, cs], h0b[:, cs], Act.Exp,
                                     scale=-2.0 * GAMMA, bias=eps[:])
                nc.scalar.activation(p1[:, cs], z1[:, cs], Act.Ln,
                                     bias=1.0, scale=-1.0)
                nc.scalar.activation(q1[:, cs], z1[:, cs], Act.Ln,
                                     bias=1.0, scale=1.0)
                nc.vector.scalar_tensor_tensor(L1b[:, cs], p1[:, cs], LCLAMP,
                                               q1[:, cs],
                                               op0=AluOpType.max,
                                               op1=AluOpType.subtract)

            # --- y0 (full, head0 pre-scaled 1/8) + layer-1 sharded ---
            # Strictly sequential PSUM accumulation groups (no group spans
            # another); head1 gets its own tile, summed on DVE.
            Y = ps.tile([C, B], F32)
            for k in range(KCH):
                nc.tensor.matmul(Y[:], hd0t[:, k * C:(k + 1) * C],
                                 h0b[:, k * B:(k + 1) * B],
                                 start=(k == 0), stop=(k == KCH - 1))
            S1 = ps.tile([OSH, B], F32)
            HL1 = ps.tile([OSH, B], F32)
            for k in range(KCH):
                nc.tensor.matmul(HL1[:], w1t[:, k * OSH:(k + 1) * OSH],
                                 h0b[:, k * B:(k + 1) * B],
                                 start=(k == 0), stop=(k == KCH - 1))
            for k in range(KCH):
                nc.tensor.matmul(S1[:], m1t[:, k * OSH:(k + 1) * OSH],
                                 L1b[:, k * B:(k + 1) * B],
                                 start=(k == 0), stop=(k == KCH - 1))

            if dbg:
                nc.sync.dma_start(Lb_o[:, :], Lb[:])
                nc.sync.dma_start(h0_o[:, :], h0b[:])
                S0c = sb.tile([128, KCH * B], F32)
                HL0c = sb.tile([128, KCH * B], F32)
                for h in range(2):
                    cs = slice(h * HB, (h + 1) * HB)
                    nc.vector.tensor_copy(S0c[:, cs], S0h[h][:])
                    nc.vector.tensor_copy(HL0c[:, cs], HL0h[h][:])
                nc.sync.dma_start(S0_o[:, :], S0c[:])
                nc.sync.dma_start(HL0_o[:, :], HL0c[:])
            d1 = sb.tile([OSH, B], F32)
            nc.scalar.activation(d1[:], S1[:], Act.Exp)
            h1b = sb.tile([OSH, B], BF16)
            nc.vector.scalar_tensor_tensor(h1b[:], HL1[:], 0.0, d1[:],
                                           op0=AluOpType.max,
                                           op1=AluOpType.mult)
            Y1 = ps.tile([C, B], F32)
            nc.tensor.matmul(Y1[:], hd1t[:], h1b[:], start=True, stop=True)
            yo = sb.tile([C, B], F32)
            nc.vector.tensor_copy(yo[:], Y[:])
            nc.vector.tensor_tensor(yo[:], yo[:], Y1[:], op=AluOpType.add)
            nc.sync.dma_start(y_part[:, :], yo[:])
    nc.compile()
    return nc


def _omajor_blocks(m: np.ndarray) -> np.ndarray:
    """[1024 out, 1024 in] -> [128, 64*128]: lhsT block (o,k) at col
    (o*8+k)*128; arr[p, (o*8+k)*128+j] = m[o*128+j, k*128+p]."""
    return np.ascontiguousarray(
        m.reshape(KCH, 128, KCH, 128).transpose(3, 0, 2, 1).reshape(128, -1))


def _chunk_major(mat_t: np.ndarray) -> np.ndarray:
    """[1024, cols] -> [128, 8*cols]: row-chunk k lands at col offset k*cols."""
    rows, cols = mat_t.shape
    assert rows == KCH * 128
    return np.ascontiguousarray(
        mat_t.reshape(KCH, 128, cols).transpose(1, 0, 2).reshape(128, KCH * cols)
    )


def kernel(x, v0, fc0, head0, v1, fc1, head1):
    nc = _CACHE.get("nc")
    if nc is None:
        nc = _CACHE["nc"] = _build()

    x = np.asarray(x, np.float32)
    m0 = (np.asarray(v0) > 0).astype(np.float32)
    w0 = (np.asarray(fc0, np.float32) * m0).astype(BF16_NP)
    m1 = (np.asarray(v1) > 0).astype(np.float32)
    w1 = (np.asarray(fc1, np.float32) * m1).astype(BF16_NP)

    xT = _chunk_major(np.ascontiguousarray(x.T)).astype(BF16_NP)
    m0f = _omajor_blocks(m0).astype(BF16_NP)
    w0f = _omajor_blocks(w0.astype(np.float32)).astype(BF16_NP)
    hd0f = _chunk_major(
        np.ascontiguousarray((np.asarray(head0, np.float32) / N_CORES).T)
    ).astype(BF16_NP)

    in_maps = []
    for c in range(N_CORES):
        sl = slice(c * OSH, (c + 1) * OSH)
        m1s = _chunk_major(np.ascontiguousarray(m1[sl].T)).astype(BF16_NP)
        w1s = _chunk_major(
            np.ascontiguousarray(w1[sl].T.astype(np.float32))).astype(BF16_NP)
        hd1s = np.ascontiguousarray(
            np.asarray(head1, np.float32)[:, sl].T).astype(BF16_NP)
        tail = np.concatenate([m1s, w1s, hd0f, hd1s], axis=1)
        in_maps.append({"xT": xT, "m0f": m0f, "w0f": w0f, "tail": tail})

    kwargs = {}
    if PROFILE["enable"]:
        kwargs = {"trace": True, **PROFILE["trace_kwargs"]}
    res = run_bass_kernel_spmd(nc, in_maps, core_ids=list(range(N_CORES)),
                               **kwargs)
    if PROFILE["enable"]:
        PROFILE["runs"].append(res)
    y = np.zeros((C, B), np.float32)
    for c in range(N_CORES):
        y += res.results[c]["y_part"]
    return np.ascontiguousarray(y.T).astype(np.float32)
